# revision 1
# baseline (speedup 1.0000x reference)
"""Trainium2 Bass kernel for nn_Encoder_HieStackedCorr.

Math (per batch element, Vmat [N=256, V=2048]):
  W1 = weight_norm(U1_v, U1_g); W2 = weight_norm(U2_v, U2_g)   (host, O(params))
  rightT = relu(W1 @ Vmat.T + b1)   [LR, N]
  leftT  = relu(W2 @ Vmat.T + b2)   [LR, N]
  diag[n] = sum_k leftT[k,n]*rightT[k,n];  d = rsqrt(diag + 1e-6)
  s[k] = sum_n d[n] leftT[k,n]
  t[m] = sum_k s[k] rightT[k,m]
  c[m] = (1 + 1/N) - d[m]*t[m]/N          (= mean_n of the uncorr matrix)
  feats[v] = sum_m c[m] Vmat[m,v]
  x = feats @ W_lin.T                      [B, E]
  (b_lin cancels in train-mode BatchNorm; BN epilogue on host, O(B*E))

Sharding: data-parallel over batch B=64 across 8 cores (8 per core);
all params replicated. Each core returns x_shard [8, 1024]; host
gathers and applies the exact batch-global BatchNorm.

Sync discipline: walrus allows at most ONE sync-wait per engine
instruction. Cross-engine clocks are advanced explicitly:
  - PE observes other engines via dummy `ldweights` reads ("sink").
  - DVE/ACT observe other engines via tiny copies into one-off
    never-reused [1,1] tiles ("touch").
With every foreign tick pre-observed, each real instruction carries at
most one wait (usually its own-engine slot-WAW or one data sem).
"""

import os
import numpy as np
from contextlib import ExitStack

import concourse.bass as bass
import concourse.bacc as bacc
import concourse.tile as tile
from concourse import mybir
from concourse.bass_utils import run_bass_kernel_spmd

B, N, V, LR, E = 64, 256, 2048, 64, 1024
NCORES = 8
BC = B // NCORES          # batches per core
NCH = V // 128            # 16 v-chunks
MH = N // 128             # 2 m-chunks of n/m axis
F32 = mybir.dt.float32

# matmul/transpose dtype knobs (float32 = exact, float32r = fast ~TF32)
_DTMAP = {"f32": mybir.dt.float32, "f32r": mybir.dt.float32r}
MM_DT = _DTMAP[os.environ.get("K_MM_DT", "f32")]
TP_DT = _DTMAP[os.environ.get("K_TP_DT", "f32")]


def _mm(ap):
    return ap.bitcast(MM_DT) if MM_DT != F32 else ap


def _tp(ap):
    return ap.bitcast(TP_DT) if TP_DT != F32 else ap


def build_kernel_a():
    nc = bacc.Bacc()
    vm = nc.declare_dram_parameter("vm", [BC, N, V], F32, isOutput=False)
    wcombT = nc.declare_dram_parameter("wcombT", [V, 128], F32, isOutput=False)
    bcomb = nc.declare_dram_parameter("bcomb", [128, 1], F32, isOutput=False)
    feats_out = nc.declare_dram_parameter("feats_out", [BC, V], F32, isOutput=True)

    with tile.TileContext(nc) as tc:
        _body_a(tc, vm, wcombT, bcomb, feats_out)
    nc.finalize()
    return nc


def build_kernel_b():
    nc = bacc.Bacc()
    feats_in = nc.declare_dram_parameter("feats_in", [BC, V], F32, isOutput=False)
    wlinT = nc.declare_dram_parameter("wlinT", [V, E], F32, isOutput=False)
    xout = nc.declare_dram_parameter("xout", [BC, E], F32, isOutput=True)

    with tile.TileContext(nc) as tc:
        _body_b(tc, feats_in, wlinT, xout)
    nc.finalize()
    return nc


def _body_b(tc, feats_in, wlinT, xout):
    nc = tc.nc
    with ExitStack() as ctx:
        consts = ctx.enter_context(tc.tile_pool(name="bconsts", bufs=1))
        ident = consts.tile([128, 128], F32)
        nc.gpsimd.memset(ident, 0.0)
        nc.gpsimd.affine_select(
            out=ident, in_=ident,
            compare_op=mybir.AluOpType.not_equal,
            fill=1.0, base=0, pattern=[[-1, 128]], channel_multiplier=1,
        )
        feats_sb = consts.tile([BC, V], F32)
        nc.sync.dma_start(out=feats_sb, in_=feats_in[:, :])
        wlin_sb = consts.tile([128, NCH, E], F32)
        nc.sync.dma_start(
            out=wlin_sb, in_=wlinT.rearrange("(c p) e -> p c e", p=128)
        )
        ftT_sb = consts.tile([128, NCH * BC], F32)
        ftT_cb = ftT_sb.rearrange("p (c bb) -> p c bb", bb=BC)
        tpool = ctx.enter_context(tc.tile_pool(name="btouch", bufs=1))
        ftp_pool = ctx.enter_context(
            tc.tile_pool(name="ft_ps", bufs=2, space="PSUM"))
        xps_pool = ctx.enter_context(
            tc.tile_pool(name="bx_ps", bufs=1, space="PSUM"))

        nc.tensor.ldweights(ident[0:1, 0:1].bitcast(mybir.dt.bfloat16))
        nc.tensor.ldweights(feats_sb[0:1, 0:1].bitcast(mybir.dt.bfloat16))
        for c in range(NCH):
            ft_ps_full = ftp_pool.tile([128, 512], F32, tag="ftps")
            ft_ps = ft_ps_full[:, 0:BC]
            nc.tensor.transpose(
                out=_tp(ft_ps),
                in_=_tp(feats_sb[:, c * 128 : (c + 1) * 128]),
                identity=_tp(ident[0:BC, 0:BC]),
            )
            t = tpool.tile([1, 1], F32, name=f"btch{c}", tag=f"btch{c}")
            nc.vector.tensor_copy(out=t, in_=ft_ps[0:1, 0:1])
            nc.vector.tensor_copy(out=ftT_cb[:, c, :], in_=ft_ps)
        nc.tensor.ldweights(
            ftT_cb[0:1, NCH - 1, 0:1].bitcast(mybir.dt.bfloat16))
        nc.tensor.ldweights(wlin_sb[0:1, 0, 0:1].bitcast(mybir.dt.bfloat16))
        x_ps = xps_pool.tile([BC, E], F32, tag="xps")
        for c in range(NCH):
            for seg in range(E // 512):
                nc.tensor.matmul(
                    out=x_ps[:, seg * 512 : (seg + 1) * 512],
                    lhsT=_mm(ftT_cb[:, c, :]),
                    rhs=_mm(wlin_sb[:, c, seg * 512 : (seg + 1) * 512]),
                    start=(c == 0), stop=(c == NCH - 1),
                )
        tx = tpool.tile([1, 1], F32, name="btchx", tag="btchx")
        nc.scalar.activation(
            out=tx, in_=x_ps[0:1, 0:1], func=mybir.ActivationFunctionType.Copy
        )
        x_sb = consts.tile([BC, E], F32)
        nc.scalar.activation(
            out=x_sb, in_=x_ps, func=mybir.ActivationFunctionType.Copy
        )
        nc.gpsimd.dma_start(out=xout[:, :], in_=x_sb)


def _body_a(tc, vm, wcombT, bcomb, feats_out):
    nc = tc.nc

    with ExitStack() as ctx:
        consts = ctx.enter_context(tc.tile_pool(name="consts", bufs=1))
        ident = consts.tile([128, 128], F32)
        nc.gpsimd.memset(ident, 0.0)
        nc.gpsimd.affine_select(
            out=ident, in_=ident,
            compare_op=mybir.AluOpType.not_equal,
            fill=1.0, base=0, pattern=[[-1, 128]], channel_multiplier=1,
        )
        ones_col = consts.tile([128, 1], F32)
        nc.vector.memset(ones_col, 1.0)
        ones_row = consts.tile([1, 128], F32)
        nc.vector.memset(ones_row, 1.0)
        eps_t = consts.tile([1, 1], F32)
        nc.vector.memset(eps_t, 1e-6)
        bcomb_sb = consts.tile([128, 1], F32)
        nc.sync.dma_start(out=bcomb_sb, in_=bcomb[:, :])
        wcomb_sb = consts.tile([128, NCH, 128], F32)
        nc.sync.dma_start(
            out=wcomb_sb, in_=wcombT.rearrange("(c p) k -> p c k", p=128)
        )
        vmat_pool = ctx.enter_context(tc.tile_pool(name="vmat", bufs=8))
        vt_pool = ctx.enter_context(tc.tile_pool(name="vt", bufs=16))
        work = ctx.enter_context(tc.tile_pool(name="work", bufs=2))
        tpool = ctx.enter_context(tc.tile_pool(name="touch", bufs=1))
        tcnt = [0]

        def sink(ap):
            """PE observes ap's producer: dummy ldweights (no output, 1 wait)."""
            nc.tensor.ldweights(ap.bitcast(mybir.dt.bfloat16))

        def dve_touch(ap):
            """DVE observes ap's producer: tiny copy into a one-off tile."""
            tcnt[0] += 1
            t = tpool.tile([1, 1], F32, name=f"tch{tcnt[0]}", tag=f"tch{tcnt[0]}")
            nc.vector.tensor_copy(out=t, in_=ap)

        def act_touch(ap):
            """ACT observes ap's producer: tiny copy into a one-off tile."""
            tcnt[0] += 1
            t = tpool.tile([1, 1], F32, name=f"tch{tcnt[0]}", tag=f"tch{tcnt[0]}")
            nc.scalar.activation(
                out=t, in_=ap, func=mybir.ActivationFunctionType.Copy
            )

        pdf_ctx = ExitStack()
        proj_ps = pdf_ctx.enter_context(
            tc.tile_pool(name="proj_ps", bufs=2, space="PSUM"))
        tp_ps_pool = pdf_ctx.enter_context(
            tc.tile_pool(name="tp_ps", bufs=2, space="PSUM"))
        d_ps_pool = pdf_ctx.enter_context(
            tc.tile_pool(name="d_ps", bufs=1, space="PSUM"))
        f_ps_pool = pdf_ctx.enter_context(
            tc.tile_pool(name="f_ps", bufs=2, space="PSUM"))

        # absorb const-producer waits (gpsimd identity, wcomb DMA) before use
        sink(ident[0:1, 0:1])
        sink(wcomb_sb[0:1, 0, 0:1])
        act_touch(bcomb_sb[0:1, 0:1])   # ACT observes bcomb DMA queue
        act_touch(eps_t[0:1, 0:1])      # ACT observes DVE (eps memset)

        def load_vmat(b):
            vmt = vmat_pool.tile([128, MH, V], F32, tag="vmt")
            nc.sync.dma_start(
                out=vmt, in_=vm[b].rearrange("(h p) v -> p h v", p=128)
            )
            return vmt

        def proj_phase(b, vmt, prev_sq):
            """Transposes + projection matmuls for batch b. Returns psum [128, N]:
            rows 0:64 = rightT, 64:128 = leftT (pre-bias, pre-relu)."""
            psp_full = proj_ps.tile([128, 512], F32, tag="psp")
            psp = psp_full[:, 0:N]
            sink(vmt[0:1, 0, 0:1])  # PE observes this batch's vmt DMA
            prev = None  # (chunk_idx, vt_sb)
            for c in range(NCH):
                if c == 1 and prev_sq is not None:
                    # PE observes ACT >= sqrt(b-2) (covers relu/relu2(b-2)
                    # reads that released this psp slot)
                    sink(prev_sq[0:1, 0:1])
                vt_p_full = tp_ps_pool.tile([128, 512], F32, tag="vt_p")
                vt_p = vt_p_full[:, 0:N]
                for h in range(MH):
                    nc.tensor.transpose(
                        out=_tp(vt_p[:, h * 128 : (h + 1) * 128]),
                        in_=_tp(vmt[:, h, c * 128 : (c + 1) * 128]),
                        identity=_tp(ident),
                    )
                if c == 0:
                    dve_touch(vt_p[0:1, 0:1])  # DVE observes PE for batch b
                vt_sb = vt_pool.tile([128, N], F32, tag="vt_sb")
                nc.vector.tensor_copy(out=vt_sb, in_=vt_p)
                if prev is not None:
                    pc, pvt = prev
                    nc.tensor.matmul(
                        out=psp, lhsT=_mm(wcomb_sb[:, pc, :]), rhs=_mm(pvt),
                        start=(pc == 0), stop=False,
                    )
                prev = (c, vt_sb)
            pc, pvt = prev
            nc.tensor.matmul(
                out=psp, lhsT=_mm(wcomb_sb[:, pc, :]), rhs=_mm(pvt),
                start=(pc == 0), stop=True,
            )
            return psp

        def df_phase(b, vmt, psp, prev_cp):
            """Per-batch vector math + feats -> feats_out row.
            Returns (sq_sb, cp_sb)."""
            act_touch(psp[0:1, 0:1])            # ACT observes PE(psp)
            if prev_cp is not None:
                # ACT observes DVE >= cp-copy(b-1): releases of this batch's
                # d_ps rotation slots are all older DVE/ACT reads
                act_touch(prev_cp[0:1, 0:1])
            # relu'd right into PSUM first, so the later left*right product
            # can mix spaces (base-partition equality only binds SBUF pairs)
            rr_ps = d_ps_pool.tile([64, N], F32, tag="dps")
            nc.scalar.activation(
                out=rr_ps, in_=psp[0:64, :],
                func=mybir.ActivationFunctionType.Relu,
                bias=bcomb_sb[0:64, :], scale=1.0,
            )
            lr_sb = work.tile([128, N], F32, tag="lr")
            nc.scalar.activation(
                out=lr_sb, in_=psp, func=mybir.ActivationFunctionType.Relu,
                bias=bcomb_sb, scale=1.0,
            )
            rightT = lr_sb[0:64, :]
            leftT = lr_sb[64:128, :]
            sink(lr_sb[0:1, 0:1])               # PE observes ACT >= relu > rr
            dve_touch(lr_sb[0:1, 0:1])          # DVE observes ACT(relu)
            dve_touch(rr_ps[0:1, 0:1])          # DVE observes ACT(relu2)
            lrprod = work.tile([64, N], F32, tag="lrprod")
            nc.vector.tensor_mul(lrprod, leftT, rr_ps)
            sink(lrprod[0:1, 0:1])              # PE observes DVE(lrprod)
            diag_ps = d_ps_pool.tile([1, N], F32, tag="dps")
            nc.tensor.matmul(
                out=diag_ps, lhsT=_mm(ones_col[0:64, :]), rhs=_mm(lrprod),
                start=True, stop=True,
            )
            act_touch(diag_ps[0:1, 0:1])        # ACT observes PE(diag)
            sq_sb = work.tile([1, N], F32, tag="sq")
            nc.scalar.activation(
                out=sq_sb, in_=diag_ps, func=mybir.ActivationFunctionType.Sqrt,
                bias=eps_t[0:1, :], scale=1.0,
            )
            dve_touch(sq_sb[0:1, 0:1])          # DVE observes ACT(sqrt)
            d_sb = work.tile([1, N], F32, tag="d")
            nc.vector.reciprocal(out=d_sb, in_=sq_sb)
            sink(sq_sb[0:1, 0:1])               # PE observes ACT(sqrt)
            sink(d_sb[0:1, 0:1])                # PE observes DVE(recip)
            dbc_ps = d_ps_pool.tile([64, N], F32, tag="dps")
            nc.tensor.matmul(
                out=dbc_ps, lhsT=_mm(ones_row[0:1, 0:64]), rhs=_mm(d_sb),
                start=True, stop=True,
            )
            dve_touch(dbc_ps[0:1, 0:1])         # DVE observes PE(dbc)
            dleft = work.tile([64, N], F32, tag="dleft")
            nc.vector.tensor_mul(dleft, leftT, dbc_ps)
            s_sb = work.tile([64, 1], F32, tag="s")
            nc.vector.reduce_sum(out=s_sb, in_=dleft, axis=mybir.AxisListType.X)
            sink(s_sb[0:1, 0:1])                # PE observes DVE(reduce)
            t_ps = d_ps_pool.tile([1, N], F32, tag="dps")
            nc.tensor.matmul(
                out=t_ps, lhsT=_mm(s_sb), rhs=_mm(rightT), start=True, stop=True
            )
            dve_touch(t_ps[0:1, 0:1])           # DVE observes PE(t)
            dt_sb = work.tile([1, N], F32, tag="dt")
            nc.vector.tensor_mul(dt_sb, d_sb, t_ps)
            c_sb = work.tile([1, N], F32, tag="c")
            nc.vector.tensor_scalar(
                out=c_sb, in0=dt_sb, scalar1=-1.0 / N, scalar2=1.0 + 1.0 / N,
                op0=mybir.AluOpType.mult, op1=mybir.AluOpType.add,
            )
            sink(c_sb[0:1, 0:1])                # PE observes DVE(c)
            cp_ps = d_ps_pool.tile([128, MH], F32, tag="dps")
            for h in range(MH):
                nc.tensor.transpose(
                    out=_tp(cp_ps[:, h : h + 1]),
                    in_=_tp(c_sb[0:1, h * 128 : (h + 1) * 128]),
                    identity=_tp(ident[0:1, 0:1]),
                )
            dve_touch(cp_ps[0:1, 0:1])          # DVE observes PE(cp)
            cp_sb = work.tile([128, MH], F32, tag="cp")
            nc.vector.tensor_copy(out=cp_sb, in_=cp_ps)
            sink(cp_sb[0:1, 0:1])               # PE observes DVE(cp copy)
            # feats[v] = sum_m c[m] Vmat[m, v], in 512-wide segments
            fstage = work.tile([1, V], F32, tag="fstage")
            for seg in range(V // 512):
                f_ps = f_ps_pool.tile([1, 512], F32, tag="fps")
                for h in range(MH):
                    nc.tensor.matmul(
                        out=f_ps,
                        lhsT=_mm(cp_sb[:, h : h + 1]),
                        rhs=_mm(vmt[:, h, seg * 512 : (seg + 1) * 512]),
                        start=(h == 0), stop=(h == MH - 1),
                    )
                dve_touch(f_ps[0:1, 0:1])       # DVE observes PE(feats seg)
                nc.vector.tensor_copy(
                    out=fstage[0:1, seg * 512 : (seg + 1) * 512], in_=f_ps
                )
            nc.gpsimd.dma_start(out=feats_out[b : b + 1, :], in_=fstage)
            return sq_sb, cp_sb

        # ---- software-pipelined batch loop: proj(b) runs while DF(b-1) drains
        vmt_prev = load_vmat(0)
        psp_prev = None
        sq_hist = [None, None]  # sq_sb handles of df(b-1), df(b-2)
        cp_prev = None
        for b in range(BC):
            psp = proj_phase(b, vmt_prev, sq_hist[1])
            vmt_cur = vmt_prev
            if b + 1 < BC:
                vmt_next = load_vmat(b + 1)
            if psp_prev is not None:
                sq_i, cp_prev = df_phase(b - 1, vmt_pp, psp_prev, cp_prev)
                sq_hist = [sq_i, sq_hist[0]]
            psp_prev, vmt_pp = psp, vmt_cur
            if b + 1 < BC:
                vmt_prev = vmt_next
        df_phase(BC - 1, vmt_pp, psp_prev, cp_prev)
        pdf_ctx.close()


_NC_CACHE = {}

# test-harness knobs (ignored by graders calling kernel() directly)
PROFILE = False
LAST_RESULT = None
LAST_RESULT_B = None


def _get_nc(which):
    if which not in _NC_CACHE:
        _NC_CACHE[which] = (
            build_kernel_a() if which == "a" else build_kernel_b()
        )
    return _NC_CACHE[which]


def kernel(**inputs):
    Vmat = np.asarray(inputs["Vmat"], dtype=np.float32)
    U1_v = np.asarray(inputs["U1_v"], dtype=np.float32)
    U1_g = np.asarray(inputs["U1_g"], dtype=np.float32)
    U1_b = np.asarray(inputs["U1_b"], dtype=np.float32)
    U2_v = np.asarray(inputs["U2_v"], dtype=np.float32)
    U2_g = np.asarray(inputs["U2_g"], dtype=np.float32)
    U2_b = np.asarray(inputs["U2_b"], dtype=np.float32)
    W_lin = np.asarray(inputs["W_lin"], dtype=np.float32)
    b_lin = np.asarray(inputs["b_lin"], dtype=np.float32)
    bn_gamma = np.asarray(inputs["bn_gamma"], dtype=np.float32)
    bn_beta = np.asarray(inputs["bn_beta"], dtype=np.float32)

    # host O(params) prep: weight-norm + packed transposed layouts
    W1 = U1_v * (U1_g / np.linalg.norm(U1_v, axis=1))[:, None]
    W2 = U2_v * (U2_g / np.linalg.norm(U2_v, axis=1))[:, None]
    wcombT = np.ascontiguousarray(np.concatenate([W1, W2], axis=0).T)  # [V, 128]
    bcomb = np.concatenate([U1_b, U2_b]).reshape(128, 1).astype(np.float32)
    wlinT = np.ascontiguousarray(W_lin.T)  # [V, E]

    nca = _get_nc("a")
    in_maps = [
        {
            "vm": np.ascontiguousarray(Vmat[i * BC : (i + 1) * BC]),
            "wcombT": wcombT,
            "bcomb": bcomb,
        }
        for i in range(NCORES)
    ]
    global LAST_RESULT, LAST_RESULT_B
    res = run_bass_kernel_spmd(nca, in_maps, list(range(NCORES)), trace=PROFILE)
    LAST_RESULT = res
    ncb = _get_nc("b")
    in_maps_b = [
        {
            "feats_in": np.ascontiguousarray(
                np.asarray(res.results[i]["feats_out"])
            ),
            "wlinT": wlinT,
        }
        for i in range(NCORES)
    ]
    res_b = run_bass_kernel_spmd(ncb, in_maps_b, list(range(NCORES)), trace=PROFILE)
    LAST_RESULT_B = res_b
    x = np.concatenate(
        [np.asarray(res_b.results[i]["xout"]) for i in range(NCORES)], axis=0
    )

    # exact batch-global BatchNorm epilogue (b_lin cancels but keep fidelity)
    x = x + b_lin
    mu = x.mean(axis=0)
    var = np.mean((x - mu) ** 2, axis=0)
    out = bn_gamma * (x - mu) / np.sqrt(var + 1e-5) + bn_beta
    return out.astype(np.float32)



# revision 13
# speedup vs baseline: 2.1078x; 2.1078x over previous
"""Trainium2 Bass kernel for nn_Encoder_HieStackedCorr (single NEFF, bf16).

Math (per batch element, Vmat [N=256, V=2048]):
  W1 = weight_norm(U1_v, U1_g); W2 = weight_norm(U2_v, U2_g)   (host, O(params))
  rightT = relu(W1 @ Vmat.T + b1)   [LR, N]
  leftT  = relu(W2 @ Vmat.T + b2)   [LR, N]
  diag[n] = sum_k leftT[k,n]*rightT[k,n];  d = rsqrt(diag + 1e-6)
  s[k] = sum_n d[n] leftT[k,n]
  t[m] = sum_k s[k] rightT[k,m]
  c[m] = (1 + 1/N) - d[m]*t[m]/N          (= mean_n of the uncorr matrix)
  ftT[v] = sum_m c[m] Vmat[m,v]            (feats, kept v-major on chip)
  x = feats @ W_lin.T                      [B, E]
  (b_lin cancels in train-mode BatchNorm; BN epilogue on host, O(B*E))

Perf design vs v1 (285us two-NEFF f32 version):
  - Vmat/weights cast to bf16 on host: transposes 2x faster (1 cyc/row),
    proj/feats/wlin matmuls 4x faster, DMA halved.
  - diag/dbc/t matmuls run as f32r (1 cyc/row at free size 256).
  - feats computed column-major (lhsT=vmt chunk, rhs=cp column) so feats.T
    accumulates on chip [128v, NCH, BC]; kills the [1,2048] psum->sbuf
    copies and feeds the final W_lin matmul directly (single NEFF).
  - vt psum->sbuf copies alternate DVE/ACT; proj matmuls run at lag-2
    behind the transposes so copies hide under PE work.
  - df-phase of batch b-1 is interleaved into proj phase of batch b in
    small groups so cross-engine latency hides behind PE matmuls.

Sync discipline: walrus allows at most ONE sync-wait per engine
instruction. Cross-engine clocks are advanced explicitly:
  - PE observes other engines via dummy `ldweights` reads ("sink").
  - DVE/ACT observe other engines via tiny copies into one-off
    never-reused [1,1] tiles ("touch").
With every foreign tick pre-observed, each real instruction carries at
most one wait (usually its own-engine slot-WAW or one data sem).
"""

import numpy as np
from contextlib import ExitStack

import ml_dtypes

import concourse.bass as bass
import concourse.bacc as bacc
import concourse.tile as tile
from concourse import mybir
from concourse.bass_utils import run_bass_kernel_spmd

B, N, V, LR, E = 64, 256, 2048, 64, 1024
NCORES = 8
BC = B // NCORES          # batches per core
NCH = V // 128            # 16 v-chunks
MH = N // 128             # 2 m-chunks of n/m axis
NPAIR = NCH // 2          # 8 chunk-pairs per batch
F32 = mybir.dt.float32
F32R = mybir.dt.float32r
BF16 = mybir.dt.bfloat16

NP_BF16 = np.dtype(ml_dtypes.bfloat16)


def _r(ap):
    """f32 -> f32r bitcast for fast (tf32-ish) matmul on fp32 data."""
    return ap.bitcast(F32R)


def build_kernel():
    nc = bacc.Bacc()
    vm = nc.declare_dram_parameter("vm", [BC, N, V], BF16, isOutput=False)
    wcombT = nc.declare_dram_parameter("wcombT", [V, 128], BF16, isOutput=False)
    bcomb = nc.declare_dram_parameter("bcomb", [128, 1], F32, isOutput=False)
    wlinT = nc.declare_dram_parameter("wlinT", [V, E], BF16, isOutput=False)
    xout = nc.declare_dram_parameter("xout", [BC, E], F32, isOutput=True)

    with tile.TileContext(nc) as tc:
        _body(tc, vm, wcombT, bcomb, wlinT, xout)
    nc.finalize()
    return nc


def _body(tc, vm, wcombT, bcomb, wlinT, xout):
    nc = tc.nc

    with ExitStack() as ctx:
        consts = ctx.enter_context(tc.tile_pool(name="consts", bufs=1))
        identf = consts.tile([128, 128], F32)
        nc.gpsimd.memset(identf, 0.0)
        nc.gpsimd.affine_select(
            out=identf, in_=identf,
            compare_op=mybir.AluOpType.not_equal,
            fill=1.0, base=0, pattern=[[-1, 128]], channel_multiplier=1,
        )
        ident = consts.tile([128, 128], BF16)
        nc.vector.tensor_copy(out=ident, in_=identf)  # DVE observes gpsimd
        ones_col_f = consts.tile([128, 1], F32)
        nc.vector.memset(ones_col_f, 1.0)
        ones_col = consts.tile([128, 1], F32R)
        nc.vector.tensor_copy(out=ones_col, in_=ones_col_f)
        ones_row_f = consts.tile([1, 128], F32)
        nc.vector.memset(ones_row_f, 1.0)
        ones_row = consts.tile([1, 128], F32R)
        nc.vector.tensor_copy(out=ones_row, in_=ones_row_f)
        one_f32 = ones_row_f[0:1, 0:1]
        eps_t = consts.tile([1, 1], F32)
        nc.vector.memset(eps_t, 1e-6)
        bcomb_sb = consts.tile([128, 1], F32)
        nc.sync.dma_start(out=bcomb_sb, in_=bcomb[:, :])
        wcomb_sb = consts.tile([128, NCH, 128], BF16)
        nc.sync.dma_start(
            out=wcomb_sb, in_=wcombT.rearrange("(c p) k -> p c k", p=128)
        )
        wlin_sb = consts.tile([128, NCH, E], BF16)
        nc.sync.dma_start(
            out=wlin_sb, in_=wlinT.rearrange("(c p) e -> p c e", p=128)
        )
        ftT_sb = consts.tile([128, NCH * BC], BF16)
        ftT_cb = ftT_sb.rearrange("p (c bb) -> p c bb", bb=BC)
        x_sb = consts.tile([BC, E], F32)

        vmat_pool = ctx.enter_context(tc.tile_pool(name="vmat", bufs=BC))
        vt_pool = ctx.enter_context(tc.tile_pool(name="vt", bufs=4))
        work = ctx.enter_context(tc.tile_pool(name="work", bufs=2))
        tpool = ctx.enter_context(tc.tile_pool(name="touch", bufs=1))
        tcnt = [0]

        def sink(ap):
            """PE observes ap's producer: dummy ldweights (no output, 1 wait)."""
            nc.tensor.ldweights(
                ap if ap.dtype not in (F32, F32R) else ap.bitcast(BF16)
            )

        def dve_touch(ap):
            """DVE observes ap's producer: tiny copy into a one-off tile."""
            tcnt[0] += 1
            t = tpool.tile([1, 1], F32, name=f"tch{tcnt[0]}", tag=f"tch{tcnt[0]}")
            nc.vector.tensor_copy(out=t, in_=ap)

        def act_touch(ap):
            """ACT observes ap's producer: tiny copy into a one-off tile."""
            tcnt[0] += 1
            t = tpool.tile([1, 1], F32, name=f"tch{tcnt[0]}", tag=f"tch{tcnt[0]}")
            nc.scalar.activation(
                out=t, in_=ap, func=mybir.ActivationFunctionType.Copy
            )

        pdf_ctx = ExitStack()
        vtps_pool = pdf_ctx.enter_context(
            tc.tile_pool(name="vt_ps", bufs=3, space="PSUM"))
        psp_pool = pdf_ctx.enter_context(
            tc.tile_pool(name="psp_ps", bufs=2, space="PSUM"))
        d_ps_pool = pdf_ctx.enter_context(
            tc.tile_pool(name="d_ps", bufs=1, space="PSUM"))
        ft_ps_pool = pdf_ctx.enter_context(
            tc.tile_pool(name="ft_ps", bufs=2, space="PSUM"))

        # absorb const-producer waits before first use
        sink(ident[0:1, 0:1])           # PE observes DVE (ident cast)
        sink(wcomb_sb[0:1, 0, 0:1])     # PE observes wcomb DMA queue
        act_touch(bcomb_sb[0:1, 0:1])   # ACT observes bcomb DMA queue
        act_touch(eps_t[0:1, 0:1])      # ACT observes DVE (eps memset)

        def load_vmat(b):
            vmt = vmat_pool.tile([128, MH, V], BF16, tag="vmt")
            nc.sync.dma_start(
                out=vmt, in_=vm[b].rearrange("(h p) v -> p h v", p=128)
            )
            return vmt

        def proj_phase(b, vmt, prev_sq, tails):
            """Transposes + projection matmuls for batch b, with df(b-1)
            op-groups (`tails`, list of lists of closures) interleaved at
            pair boundaries. Returns psum [128, N]: rows 0:64 = rightT
            pre-bias, 64:128 = leftT pre-bias."""
            psp = psp_pool.tile([128, N], F32, tag="psp")
            sink(vmt[0:1, 0, 0:1])  # PE observes this batch's vmt DMA
            tails = list(tails)
            pend = []  # [(pair_idx, vt_sb)] copies not yet consumed (lag 2)
            for ci in range(NPAIR):
                if ci == 1 and prev_sq is not None:
                    # PE observes ACT >= sqrt(b-2): covers relu(b-2) reads
                    # that released this psp slot
                    sink(prev_sq[0:1, 0:1])
                vt_p = vtps_pool.tile([128, 512], BF16, tag="vtp")
                for j in (0, 1):
                    c = 2 * ci + j
                    for h in range(MH):
                        nc.tensor.transpose(
                            out=vt_p[:, j * 256 + h * 128 : j * 256 + (h + 1) * 128],
                            in_=vmt[:, h, c * 128 : (c + 1) * 128],
                            identity=ident,
                        )
                vt_sb = vt_pool.tile([128, 512], BF16, tag="vts")
                if ci % 2 == 0:
                    nc.vector.tensor_copy(out=vt_sb, in_=vt_p)
                else:
                    nc.scalar.activation(
                        out=vt_sb, in_=vt_p,
                        func=mybir.ActivationFunctionType.Copy,
                    )
                pend.append((ci, vt_sb))
                if len(pend) > 2:
                    pc, pvt = pend.pop(0)
                    for j in (0, 1):
                        c = 2 * pc + j
                        nc.tensor.matmul(
                            out=psp, lhsT=wcomb_sb[:, c, :],
                            rhs=pvt[:, j * 256 : (j + 1) * 256],
                            start=(c == 0), stop=(c == NCH - 1),
                        )
                if tails:
                    for op in tails.pop(0):
                        op()
            for pc, pvt in pend:
                for j in (0, 1):
                    c = 2 * pc + j
                    nc.tensor.matmul(
                        out=psp, lhsT=wcomb_sb[:, c, :],
                        rhs=pvt[:, j * 256 : (j + 1) * 256],
                        start=(c == 0), stop=(c == NCH - 1),
                    )
            while tails:
                for op in tails.pop(0):
                    op()
            return psp

        def df_ops(b, vmt, psp, prev_cp, out):
            """Build df-phase op groups for batch b (vector math + featsT).
            Appends closures into `out` (list of groups); returns
            (sq_sb, cp_bf) handles created eagerly (tiles alloc now,
            instructions deferred)."""
            rr_ps = d_ps_pool.tile([64, N], F32, tag="dps")
            lr_sb = work.tile([128, N], F32R, tag="lr")
            lrprod = work.tile([64, N], F32R, tag="lrprod")
            sq_sb = work.tile([1, N], F32, tag="sq")
            d_sb = work.tile([1, N], F32R, tag="d")
            dleft = work.tile([64, N], F32, tag="dleft")
            s_sb = work.tile([64, 1], F32R, tag="s")
            dt_sb = work.tile([1, N], F32, tag="dt")
            c_sb = work.tile([1, N], F32, tag="c")
            cp_bf = work.tile([128, MH], BF16, tag="cp")

            def g1():
                act_touch(psp[0:1, 0:1])            # ACT observes PE(psp)
                if prev_cp is not None:
                    # ACT observes DVE >= cp-copy(b-1): releases of this
                    # batch's d_ps rotation slots are all older DVE reads
                    act_touch(prev_cp[0:1, 0:1])
                # relu'd right into PSUM first, so the later left*right
                # product can mix spaces (base-partition equality only
                # binds SBUF pairs)
                nc.scalar.activation(
                    out=rr_ps, in_=psp[0:64, :],
                    func=mybir.ActivationFunctionType.Relu,
                    bias=bcomb_sb[0:64, :], scale=1.0,
                )
                nc.scalar.activation(
                    out=lr_sb, in_=psp, func=mybir.ActivationFunctionType.Relu,
                    bias=bcomb_sb, scale=1.0,
                )
                dve_touch(lr_sb[0:1, 0:1])          # DVE observes ACT(relu)
                dve_touch(rr_ps[0:1, 0:1])          # DVE observes ACT(relu2)
                nc.vector.tensor_mul(lrprod, lr_sb[64:128, :], rr_ps)

            def g2():
                sink(lr_sb[0:1, 0:1])               # PE observes ACT(relu)
                sink(lrprod[0:1, 0:1])              # PE observes DVE(lrprod)
                diag_ps = d_ps_pool.tile([1, N], F32, tag="dps")
                nc.tensor.matmul(
                    out=diag_ps, lhsT=ones_col[0:64, :], rhs=lrprod,
                    start=True, stop=True,
                )
                act_touch(diag_ps[0:1, 0:1])        # ACT observes PE(diag)
                nc.scalar.activation(
                    out=sq_sb, in_=diag_ps,
                    func=mybir.ActivationFunctionType.Sqrt,
                    bias=eps_t[0:1, :], scale=1.0,
                )
                dve_touch(sq_sb[0:1, 0:1])          # DVE observes ACT(sqrt)
                with nc.allow_low_precision(
                    reason="d consumed by f32r matmul; tf32 rounding ok"
                ):
                    nc.vector.reciprocal(out=d_sb, in_=sq_sb)

            def g3():
                sink(sq_sb[0:1, 0:1])               # PE observes ACT(sqrt)
                sink(d_sb[0:1, 0:1])                # PE observes DVE(recip)
                dbc_ps = d_ps_pool.tile([64, N], F32, tag="dps")
                nc.tensor.matmul(
                    out=dbc_ps, lhsT=ones_row[0:1, 0:64], rhs=d_sb,
                    start=True, stop=True,
                )
                dve_touch(dbc_ps[0:1, 0:1])         # DVE observes PE(dbc)
                nc.vector.tensor_mul(dleft, lr_sb[64:128, :], dbc_ps)
                with nc.allow_low_precision(
                    reason="s accumulates f32; f32r out only rounds mantissa"
                ):
                    nc.vector.reduce_sum(
                        out=s_sb, in_=dleft, axis=mybir.AxisListType.X
                    )

            def g4():
                sink(s_sb[0:1, 0:1])                # PE observes DVE(reduce)
                t_ps = d_ps_pool.tile([1, N], F32, tag="dps")
                nc.tensor.matmul(
                    out=t_ps, lhsT=s_sb, rhs=lr_sb[0:64, :],
                    start=True, stop=True,
                )
                dve_touch(t_ps[0:1, 0:1])           # DVE observes PE(t)
                nc.vector.tensor_mul(dt_sb, d_sb, t_ps)
                nc.vector.tensor_scalar(
                    out=c_sb, in0=dt_sb, scalar1=-1.0 / N, scalar2=1.0 + 1.0 / N,
                    op0=mybir.AluOpType.mult, op1=mybir.AluOpType.add,
                )

            def g5():
                sink(c_sb[0:1, 0:1])                # PE observes DVE(c)
                cp_ps = d_ps_pool.tile([128, MH], F32, tag="dps")
                for h in range(MH):
                    nc.tensor.transpose(
                        out=cp_ps[:, h : h + 1],
                        in_=c_sb[0:1, h * 128 : (h + 1) * 128],
                        identity=one_f32[0:1, 0:1],
                    )
                dve_touch(cp_ps[0:1, 0:1])          # DVE observes PE(cp)
                nc.vector.tensor_copy(out=cp_bf, in_=cp_ps)

            def g6():
                # featsT column-major: ftT[v, b] = sum_m vmt[m, v] c[m]
                sink(cp_bf[0:1, 0:1])               # PE observes DVE(cp copy)
                ft_ps = ft_ps_pool.tile([128, NCH], F32, tag="ftps")
                for c in range(NCH):
                    for h in range(MH):
                        nc.tensor.matmul(
                            out=ft_ps[:, c : c + 1],
                            lhsT=vmt[:, h, c * 128 : (c + 1) * 128],
                            rhs=cp_bf[:, h : h + 1],
                            start=(h == 0), stop=(h == MH - 1),
                        )
                dve_touch(ft_ps[0:1, 0:1])          # DVE observes PE(feats)
                nc.vector.tensor_copy(out=ftT_cb[:, :, b], in_=ft_ps)

            out.extend([[g1], [g2], [g3], [g4], [g5], [g6]])
            return sq_sb, cp_bf

        # ---- software-pipelined batch loop: proj(b) runs while DF(b-1)
        # ops interleave into its pair boundaries
        vmts = [load_vmat(b) for b in range(BC)]
        psp_prev = None
        sq_hist = [None, None]  # sq_sb handles of df(b-1), df(b-2)
        cp_prev = None
        for b in range(BC):
            tails = []
            if psp_prev is not None:
                sq_i, cp_prev = df_ops(b - 1, vmts[b - 1], psp_prev, cp_prev, tails)
                sq_hist = [sq_i, sq_hist[0]]
            psp_prev = proj_phase(b, vmts[b], sq_hist[1], tails)
        tails = []
        sq_i, cp_prev = df_ops(BC - 1, vmts[BC - 1], psp_prev, cp_prev, tails)
        for grp in tails:
            for op in grp:
                op()

        # ---- final projection: x = feats @ W_lin.T  [BC, E]
        pdf_ctx.close()
        xps_ctx = ExitStack()
        xps_pool = xps_ctx.enter_context(
            tc.tile_pool(name="x_ps", bufs=1, space="PSUM"))
        sink(ftT_cb[0:1, NCH - 1, BC - 1 : BC])  # PE observes DVE ftT(b=7)
        sink(wlin_sb[0:1, 0, 0:1])               # PE observes wlin DMA
        x_ps = xps_pool.tile([BC, E], F32, tag="xps")
        for c in range(NCH):
            for seg in range(E // 512):
                nc.tensor.matmul(
                    out=x_ps[:, seg * 512 : (seg + 1) * 512],
                    lhsT=ftT_cb[:, c, :],
                    rhs=wlin_sb[:, c, seg * 512 : (seg + 1) * 512],
                    start=(c == 0), stop=(c == NCH - 1),
                )
        nc.vector.tensor_copy(out=x_sb, in_=x_ps)  # carries PE wait
        nc.gpsimd.dma_start(out=xout[:, :], in_=x_sb)
        xps_ctx.close()


_NC_CACHE = {}

# test-harness knobs (ignored by graders calling kernel() directly)
PROFILE = False
LAST_RESULT = None
LAST_RESULT_B = None


def _get_nc():
    if "k" not in _NC_CACHE:
        _NC_CACHE["k"] = build_kernel()
    return _NC_CACHE["k"]


def kernel(**inputs):
    Vmat = np.asarray(inputs["Vmat"], dtype=np.float32)
    U1_v = np.asarray(inputs["U1_v"], dtype=np.float32)
    U1_g = np.asarray(inputs["U1_g"], dtype=np.float32)
    U1_b = np.asarray(inputs["U1_b"], dtype=np.float32)
    U2_v = np.asarray(inputs["U2_v"], dtype=np.float32)
    U2_g = np.asarray(inputs["U2_g"], dtype=np.float32)
    U2_b = np.asarray(inputs["U2_b"], dtype=np.float32)
    W_lin = np.asarray(inputs["W_lin"], dtype=np.float32)
    b_lin = np.asarray(inputs["b_lin"], dtype=np.float32)
    bn_gamma = np.asarray(inputs["bn_gamma"], dtype=np.float32)
    bn_beta = np.asarray(inputs["bn_beta"], dtype=np.float32)

    # host O(params) prep: weight-norm + packed transposed bf16 layouts
    W1 = U1_v * (U1_g / np.linalg.norm(U1_v, axis=1))[:, None]
    W2 = U2_v * (U2_g / np.linalg.norm(U2_v, axis=1))[:, None]
    wcombT = np.ascontiguousarray(
        np.concatenate([W1, W2], axis=0).T
    ).astype(NP_BF16)  # [V, 128]
    bcomb = np.concatenate([U1_b, U2_b]).reshape(128, 1).astype(np.float32)
    wlinT = np.ascontiguousarray(W_lin.T).astype(NP_BF16)  # [V, E]
    vm_bf = Vmat.astype(NP_BF16)

    nck = _get_nc()
    in_maps = [
        {
            "vm": np.ascontiguousarray(vm_bf[i * BC : (i + 1) * BC]),
            "wcombT": wcombT,
            "bcomb": bcomb,
            "wlinT": wlinT,
        }
        for i in range(NCORES)
    ]
    global LAST_RESULT, LAST_RESULT_B
    res = run_bass_kernel_spmd(nck, in_maps, list(range(NCORES)), trace=PROFILE)
    LAST_RESULT = res
    LAST_RESULT_B = None
    x = np.concatenate(
        [np.asarray(res.results[i]["xout"]) for i in range(NCORES)], axis=0
    )

    # exact batch-global BatchNorm epilogue (b_lin cancels but keep fidelity)
    x = x + b_lin
    mu = x.mean(axis=0)
    var = np.mean((x - mu) ** 2, axis=0)
    out = bn_gamma * (x - mu) / np.sqrt(var + 1e-5) + bn_beta
    return out.astype(np.float32)
